# revision 17
# baseline (speedup 1.0000x reference)
"""Trainium2 Bass kernel for nn_AdaFeatBlock (modulated deformable-conv block).

Sharding: data-parallel over batch — 8 samples -> 8 NeuronCores, all weights
replicated; each core computes its sample end-to-end, host stacks outputs.

Per-core pipeline (one sample, x [64,128,128]):
  1. x -> bf16 padded layout x_sb: partition h*64+c; free = 76 rows
     (half-rows -6..69) x 138 cols (-4..133), zero borders.
  2. offset/mask 3x3 conv = 9 shifted matmuls, block-diagonal [128, 54]
     lhsT, PSUM-accumulated -> om [54, 8192] (per half: off_y k0..8 |
     off_x k0..8 | mask k0..8).
  3. Quad tables for ap_gather, CHANNEL-PAIRED (d=4): partition group
     g = st*2+h (st = row-half of the image half, h = image half) holds
     channel-pairs (c, c+32) of its stream; element = 16B = 2x2 pixel
     quad for c and c+32. Window per stream: 40 rows -> NBLK =
     4 classes x 20 x 68 = 5440 (fits ap_gather's 2^15-word limit).
     Because each 16-partition Q7 core uses its own index list, one
     gather SLOT serves FOUR samples (A-h0, A-h1, B-h0, B-h1):
     36864 slots instead of 73728. ap_gather is per-slot bound
     (~29ns d=2 vs ~31ns d=4 measured), so this halves the gather.
  4. Coordinate math in one pass on [128, 2048] tiles, partition
     P = 32*cc + h*9 + k; floor via the 2^23 trick; idx written in
     wrapped order, bounced via DRAM into per-group idxt streams;
     corner weights (x mask) -> wq [4q x 18(h,k) rows, 8192].
  5. Gathers as k-pairs per 1024-px chunk (20 calls); per (k, q, sub):
     2 selector matmuls broadcast wq rows -> mq [128, 512] PSUM; one
     DVE mult -> ht2 [128, 2(lo/hi), 512] bf16; deformable conv =
     K=64 lo/hi matmuls with 4-way block-diagonal channel weights,
     PSUM-accumulated over (9k x 4q x 2w) -> + b_dc -> out.

Bottleneck: ap_gather ~31ns/slot x 36864 ~ 1.14 ms. (SWDGE dma_gather
InstDMAGatherAnt crashes this firmware — mlp/attnmlp library loads fine
but the gather kills the exec unit; native indirect_dma_start works but
is 56ns/descriptor Q7-bound. Both measured on HW.)
"""

import numpy as np

import concourse.bass as bass
import concourse.tile as tile
from concourse import mybir
from concourse.bass_utils import run_bass_kernel_spmd
from concourse import library_config
from concourse.library_overlay import lower_extended_insts
from concourse.vector_clock import ScopedClock

AF = mybir.ActivationFunctionType
ALU = mybir.AluOpType
DT = mybir.dt

B, C, H, W = 8, 64, 128, 128
O = 64
K = 3
KF = 9
NCORES = 8
HALF = H // 2
NPIX = H * W // 2              # 8192 pixels per half
ROWS_ST = 76                   # stored rows per half (-6..69)
PITCH = 138                    # stored cols (-4..133)
RY = 20                        # y-block starts per parity per stream window
RX = 68                        # x-block starts per parity
NCLS = RY * RX                 # 1360
NBLK = 4 * NCLS                # 5440
NSLOT = 4 * KF * 1024          # 36864 slots (each = 4 samples)
SUB = 512


def _install_compat():
    """This walrus build accepts at most ONE sync-wait per instruction."""
    if getattr(tile.TileContext, "_adafeat_patched", False):
        return
    _orig_lower = tile.TileContext._lower_ordered_insts

    def _split_waits(nc, ordered):
        for insts in ordered.values():
            new_insts = []
            for inst in insts:
                si = inst.sync_info
                if si is not None and si.on_wait and len(si.on_wait) > 1:
                    waits = list(si.on_wait)
                    for w in waits[:-1]:
                        nop = mybir.InstNoOp(name=f"I-{nc.next_id()}", ins=[], outs=[])
                        nop.engine = inst.engine
                        nop.sync_info = mybir.SyncInfo(on_wait=[w], on_update=[])
                        new_insts.append(nop)
                    inst.sync_info = mybir.SyncInfo(
                        on_wait=[waits[-1]], on_update=list(si.on_update)
                    )
                new_insts.append(inst)
            insts[:] = new_insts

    def _lower_split(self, ordered):
        _split_waits(self.nc, ordered)
        return _orig_lower(self, ordered)

    def _drain_split(self, tick_clock, wait_clock):
        carrier = self.nc.sync.nop(nofuse=True)
        wait_clock.add_sem_waits(
            carrier.ins, ScopedClock({None: tick_clock.global_clock})
        )
        si = carrier.ins.sync_info
        if si is not None and si.on_wait and len(si.on_wait) > 1:
            waits = list(si.on_wait)
            carrier.ins.sync_info = mybir.SyncInfo(
                on_wait=waits[:1], on_update=list(si.on_update)
            )
            for w in waits[1:]:
                extra = self.nc.sync.nop(nofuse=True)
                extra.ins.sync_info = mybir.SyncInfo(on_wait=[w], on_update=[])
        self.nc.sync.drain()
        self.nc.all_engine_barrier()
        popped = self.nc._tile_sem_poison_stack.pop()
        assert popped is self._sem_poison
        self.nc.clear_and_free_semaphores(list(self.sems.allocated().values()))
        self.nc.all_engine_barrier()

    tile.TileContext._lower_ordered_insts = _lower_split
    tile.TileContext._drain_and_barrier = _drain_split
    tile.TileContext._adafeat_patched = True


def _emit(nc, tc, x_ext, out_ext, lom_ext, bom2_ext, sel_ext, wdup_ext, bdc2_ext):
    _iotas = []

    with tc.tile_pool(name="persist", bufs=1) as persist:
        wq = persist.tile([128, NPIX], DT.bfloat16)
        idxt = persist.tile([128, NSLOT // 16], DT.int16)
        wdup = persist.tile([128, KF * 2 * 128], DT.bfloat16)
        sel = persist.tile([128, KF * 4 * 64], DT.bfloat16)
        bdc_t = persist.tile([128, 1], DT.float32)
        # channel-paired quad tables: [128, NBLK, 4 f32] = [.., 8 bf16]
        qtab = persist.tile([128, NBLK * 8], DT.bfloat16)
        xpool = tc.tile_pool(name="xp", bufs=1)
        xp = xpool.__enter__()
        x_sb = xp.tile([128, ROWS_ST * PITCH], DT.bfloat16)

        x3 = lambda: x_sb[:].rearrange("p (r c) -> p r c", c=PITCH)

        # ======== phase 1: load x f32 via HWDGE, convert to bf16 on DVE
        nc.gpsimd.memset(x_sb[:], 0.0)
        nc.gpsimd.memset(wq[:], 0.0)
        xv = x_ext[:]
        with tc.tile_pool(name="xs", bufs=1) as xsp:
            xstage = xsp.tile([128, HALF * W], DT.float32)
            xs3 = xstage[:].rearrange("p (r c) -> p r c", c=W)
            for h in range(2):
                nc.sync.dma_start(
                    out=xstage[h * 64 : (h + 1) * 64, :],
                    in_=xv[:, h * HALF : (h + 1) * HALF, :].rearrange(
                        "c r w -> c (r w)"),
                )
            for h in range(2):
                r0 = max(0, h * HALF - 6)
                r1 = min(H - 1, h * HALF + 69)
                rloc = r0 - (h * HALF - 6)
                for sh in range(2):
                    s0 = max(r0, sh * HALF)
                    s1 = min(r1, sh * HALF + HALF - 1)
                    if s0 > s1:
                        continue
                    dl = rloc + (s0 - r0)
                    nc.vector.tensor_copy(
                        out=x3()[h * 64 : h * 64 + 64,
                                 dl : dl + (s1 - s0 + 1), 4 : 4 + W],
                        in_=xs3[sh * 64 : sh * 64 + 64,
                                s0 - sh * HALF : s1 - sh * HALF + 1, :],
                    )

        # ======== phase 2: offset/mask conv -> om_dram (f32 bounce so
        # phase 3 can load arbitrary row slices via HWDGE)
        om_dram = nc.dram_tensor("om_scratch", [54, NPIX], DT.float32)
        with (
            tc.tile_pool(name="omsb", bufs=1) as omsbp,
            tc.tile_pool(name="convw", bufs=1) as convw,
            tc.tile_pool(name="convp", bufs=2, space="PSUM") as convp,
        ):
            om = omsbp.tile([54, NPIX], DT.float32)
            lhsT_om = convw.tile([128, KF * 54], DT.bfloat16)
            nc.sync.dma_start(out=lhsT_om[:], in_=lom_ext[:])
            bom_t = convw.tile([54, 1], DT.float32)
            nc.sync.dma_start(out=bom_t[:], in_=bom2_ext[:])

            rows_per_sub = SUB // W  # 4
            for cb in range(NPIX // SUB):
                pt = convp.tile([54, SUB], DT.float32, tag="cpt")
                r0 = cb * rows_per_sub
                for i, (dy, dx) in enumerate(
                    (dy, dx) for dy in range(3) for dx in range(3)
                ):
                    rhs = x3()[:, 6 + r0 + dy - 1 : 6 + r0 + dy - 1 + rows_per_sub,
                               3 + dx : 3 + dx + W]
                    nc.tensor.matmul(
                        out=pt[:], lhsT=lhsT_om[:, i * 54 : (i + 1) * 54], rhs=rhs,
                        start=(i == 0), stop=(i == 8),
                    )
                nc.vector.tensor_scalar(
                    out=om[:, cb * SUB : (cb + 1) * SUB], in0=pt[:],
                    scalar1=bom_t[:, 0:1], scalar2=None, op0=ALU.add,
                )
            for cb in range(4):
                nc.sync.dma_start(
                    out=om_dram[:, cb * 2048 : (cb + 1) * 2048],
                    in_=om[:, cb * 2048 : (cb + 1) * 2048])

        # xq staging for the quad-table fills: channel-paired, per-group
        # row windows, f32 straight from DRAM (8 HWDGE DMAs, overlaps conv)
        xqpool_cm = tc.tile_pool(name="xq", bufs=1)
        xqp = xqpool_cm.__enter__()
        xqf = xqp.tile([128, 2 * 44 * PITCH], DT.float32)
        x4 = xqf[:].rearrange("p (w r c) -> p w r c", w=2, c=PITCH)
        nc.vector.memset(xqf[:], 0.0)
        for g in range(4):
            st, h = g // 2, g % 2
            r_lo = h * 64 + 32 * st - 4
            wr0 = max(0, -r_lo)
            wr1 = min(44, 128 - r_lo)
            for w in range(2):
                nc.sync.dma_start(
                    out=x4[g * 32 : (g + 1) * 32, w, wr0:wr1, 4 : 4 + W],
                    in_=xv[w * 32 : w * 32 + 32, r_lo + wr0 : r_lo + wr1, :],
                )

        # ======== quad table build: 32 full-width strided copies from xqf
        # qtab[g*32+i, blk, w*4 + qy*2+qx] = xq[g*32+i, w, 2*by+a+qy,
        #                                       2*bx + b + qx]
        q8 = qtab[:].rearrange("p (blk q) -> p blk q", q=8)
        opi = 0
        for w in range(2):
            for a in range(2):
                for b in range(2):
                    blk0 = (a * 2 + b) * NCLS
                    for qy in range(2):
                        for qx in range(2):
                            src = x4[:, w,
                                     a + qy : a + qy + 2 * (RY - 1) + 1 : 2,
                                     b + qx : b + qx + 2 * (RX - 1) + 1 : 2]
                            dst3 = q8[:, blk0 : blk0 + NCLS,
                                      w * 4 + qy * 2 + qx :
                                      w * 4 + qy * 2 + qx + 1]
                            dst = bass.AP(
                                dst3.tensor, dst3.offset,
                                [dst3.ap[0], [RX * 8, RY], [8, RX]],
                            )
                            if opi % 2 == 0:
                                nc.scalar.activation(out=dst, in_=src,
                                                     func=AF.Copy)
                            else:
                                nc.vector.tensor_copy(out=dst, in_=src)
                            opi += 1

        xqpool_cm.__exit__(None, None, None)
        xpool.__exit__(None, None, None)

        # ======== phase 3: coordinate math, two column-half passes on
        # [128, 1024] tiles; partition P = 32*cc + h*9 + k; stream st = cc//2
        with tc.tile_pool(name="math", bufs=1) as mpool:
            idx16b = mpool.tile([128, 1024], DT.int16)
            OY = mpool.tile([128, 1024], DT.float32)
            OX = mpool.tile([128, 1024], DT.float32)
            OM = mpool.tile([128, 1024], DT.float32)
            IOTY = mpool.tile([128, 1024], DT.float32)
            IOTX = mpool.tile([128, 1024], DT.float32)
            T0 = mpool.tile([128, 1024], DT.float32)
            T1 = mpool.tile([128, 1024], DT.float32)
            T2 = mpool.tile([128, 1024], DT.float32)
            T3 = mpool.tile([128, 1024], DT.float32)
            T4 = mpool.tile([128, 1024], DT.float32)
            cst = mpool.tile([128, 8], DT.float32)

            pidx = mpool.tile([128, 4], DT.float32)
            _iotas.append(nc.gpsimd.iota(pidx[:, 0:1], pattern=[[0, 1]],
                           channel_multiplier=1,
                           allow_small_or_imprecise_dtypes=True))
            _iotas.append(nc.gpsimd.iota(IOTY[:], pattern=[[1, 8], [0, 128]],
                           channel_multiplier=0,
                           allow_small_or_imprecise_dtypes=True))
            _iotas.append(nc.gpsimd.iota(IOTX[:], pattern=[[0, 8], [1, 128]],
                           channel_multiplier=0,
                           allow_small_or_imprecise_dtypes=True))
            P128 = pidx[:, 0:1]
            hh, kk, kh3, km3, ccv, stv, cy, cx = (cst[:, i : i + 1] for i in range(8))
            t_a = pidx[:, 1:2]
            # cc = P // 32 (values 0..3)
            nc.vector.tensor_scalar(out=ccv, in0=P128, scalar1=31.5, scalar2=None, op0=ALU.is_gt)
            nc.vector.tensor_scalar(out=t_a, in0=P128, scalar1=63.5, scalar2=None, op0=ALU.is_gt)
            nc.vector.tensor_add(ccv, ccv, t_a)
            nc.vector.tensor_scalar(out=t_a, in0=P128, scalar1=95.5, scalar2=None, op0=ALU.is_gt)
            nc.vector.tensor_add(ccv, ccv, t_a)
            # st = cc // 2
            nc.vector.tensor_scalar(out=stv, in0=ccv, scalar1=1.5, scalar2=None, op0=ALU.is_gt)
            # hk = P - 32*cc ; h = hk > 8.5 ; k = hk - 9*h
            nc.vector.tensor_scalar(out=t_a, in0=ccv, scalar1=-32.0, scalar2=None, op0=ALU.mult)
            nc.vector.tensor_add(t_a, t_a, P128)
            nc.vector.tensor_scalar(out=hh, in0=t_a, scalar1=8.5, scalar2=None, op0=ALU.is_gt)
            nc.vector.tensor_scalar(out=kk, in0=hh, scalar1=-9.0, scalar2=None, op0=ALU.mult)
            nc.vector.tensor_add(kk, kk, t_a)
            nc.vector.tensor_scalar(out=kh3, in0=kk, scalar1=2.5, scalar2=None, op0=ALU.is_gt)
            nc.vector.tensor_scalar(out=t_a, in0=kk, scalar1=5.5, scalar2=None, op0=ALU.is_gt)
            nc.vector.tensor_add(kh3, kh3, t_a)
            nc.vector.tensor_scalar(out=km3, in0=kh3, scalar1=-3.0, scalar2=None, op0=ALU.mult)
            nc.vector.tensor_add(km3, km3, kk)
            # cy = 16*cc - 32*st + kh3 + 515   (sy = off_y + rowiota + cy)
            nc.vector.tensor_scalar(out=cy, in0=ccv, scalar1=16.0, scalar2=None, op0=ALU.mult)
            nc.vector.tensor_scalar(out=t_a, in0=stv, scalar1=-32.0, scalar2=None, op0=ALU.mult)
            nc.vector.tensor_add(cy, cy, t_a)
            nc.vector.tensor_add(cy, cy, kh3)
            nc.vector.tensor_scalar(out=cy, in0=cy, scalar1=515.0, scalar2=None, op0=ALU.add)
            # cx = km3 + 515
            nc.vector.tensor_scalar(out=cx, in0=km3, scalar1=515.0, scalar2=None, op0=ALU.add)

            idx_dram = nc.dram_tensor("idx_scratch", [36, 4096], DT.int16)
            base_ap = idx_dram[:]
            nc.vector.memset(OY[:], 0.0)
            nc.vector.memset(OX[:], 0.0)
            nc.vector.memset(OM[:], 0.0)
            for ch_i in range(2):
                # load offsets/mask rows (DVE: keeps GPSIMD free for gathers)
                for cc in range(4):
                    cs = slice(cc * 2048 + ch_i * 1024, cc * 2048 + ch_i * 1024 + 1024)
                    for role, dstt in ((0, OY), (1, OX), (2, OM)):
                        for h in range(2):
                            nc.sync.dma_start(
                                out=dstt[cc * 32 + h * 9 : cc * 32 + h * 9 + 9, :],
                                in_=om_dram[h * 27 + role * 9 : h * 27 + role * 9 + 9, cs],
                            )

                # sy = OY + rowiota + cy (+8 in second half); floor; frac
                nc.vector.tensor_add(T0[:], OY[:], IOTY[:])
                nc.vector.tensor_scalar(out=T0[:], in0=T0[:], scalar1=cy,
                                        scalar2=float(ch_i * 8), op0=ALU.add, op1=ALU.add)
                nc.vector.tensor_scalar(out=T2[:], in0=T0[:], scalar1=8388608.0,
                                        scalar2=-8388608.0, op0=ALU.add, op1=ALU.add)
                nc.vector.tensor_tensor(out=OY[:], in0=T2[:], in1=T0[:], op=ALU.is_gt)
                nc.vector.tensor_sub(T2[:], T2[:], OY[:])
                nc.vector.tensor_sub(OY[:], T0[:], T2[:])     # fy
                nc.vector.tensor_copy(out=T0[:], in_=T2[:])   # y0s
                # sx
                nc.vector.tensor_add(T1[:], OX[:], IOTX[:])
                nc.vector.tensor_scalar(out=T1[:], in0=T1[:], scalar1=cx,
                                        scalar2=None, op0=ALU.add)
                nc.vector.tensor_scalar(out=T2[:], in0=T1[:], scalar1=8388608.0,
                                        scalar2=-8388608.0, op0=ALU.add, op1=ALU.add)
                nc.vector.tensor_tensor(out=OX[:], in0=T2[:], in1=T1[:], op=ALU.is_gt)
                nc.vector.tensor_sub(T2[:], T2[:], OX[:])
                nc.vector.tensor_sub(OX[:], T1[:], T2[:])     # fx
                nc.vector.tensor_copy(out=T1[:], in_=T2[:])   # x0s

                # y0l = clamp(y0s-512, 0, 39); by = floor(y0l/2); a = y0l-2by
                nc.vector.tensor_scalar(out=T0[:], in0=T0[:], scalar1=-512.0,
                                        scalar2=None, op0=ALU.add)
                nc.vector.tensor_scalar(out=T0[:], in0=T0[:], scalar1=0.0, scalar2=39.0,
                                        op0=ALU.max, op1=ALU.min)
                nc.vector.tensor_scalar_mul(out=T0[:], in0=T0[:], scalar1=0.5)
                nc.vector.tensor_scalar(out=T3[:], in0=T0[:], scalar1=8388608.0,
                                        scalar2=-8388608.0, op0=ALU.add, op1=ALU.add)
                nc.vector.tensor_tensor(out=T2[:], in0=T3[:], in1=T0[:], op=ALU.is_gt)
                nc.vector.tensor_sub(T3[:], T3[:], T2[:])     # by
                nc.vector.tensor_sub(T2[:], T0[:], T3[:])     # a/2
                nc.vector.tensor_copy(out=T0[:], in_=T3[:])   # by
                # x0l = clamp(x0s-512, 0, 135); bx = floor(x0l/2); b = x0l-2bx
                nc.vector.tensor_scalar(out=T1[:], in0=T1[:], scalar1=-512.0,
                                        scalar2=None, op0=ALU.add)
                nc.vector.tensor_scalar(out=T1[:], in0=T1[:], scalar1=0.0, scalar2=135.0,
                                        op0=ALU.max, op1=ALU.min)
                nc.vector.tensor_scalar_mul(out=T1[:], in0=T1[:], scalar1=0.5)
                nc.vector.tensor_scalar(out=T4[:], in0=T1[:], scalar1=8388608.0,
                                        scalar2=-8388608.0, op0=ALU.add, op1=ALU.add)
                nc.vector.tensor_tensor(out=T3[:], in0=T4[:], in1=T1[:], op=ALU.is_gt)
                nc.vector.tensor_sub(T4[:], T4[:], T3[:])     # bx
                nc.vector.tensor_sub(T3[:], T1[:], T4[:])     # b/2
                nc.vector.tensor_copy(out=T1[:], in_=T4[:])   # bx

                # idx = a*2720 + b*1360 + by*68 + bx
                nc.vector.tensor_scalar_mul(out=T2[:], in0=T2[:], scalar1=float(2 * 2 * NCLS))
                nc.vector.tensor_scalar_mul(out=T3[:], in0=T3[:], scalar1=float(2 * NCLS))
                nc.vector.tensor_add(T2[:], T2[:], T3[:])
                nc.vector.tensor_scalar_mul(out=T0[:], in0=T0[:], scalar1=float(RX))
                nc.vector.tensor_add(T2[:], T2[:], T0[:])
                nc.vector.tensor_add(T2[:], T2[:], T1[:])

                # wrapped idx -> DRAM bounce; pc = (cc&1)*2 + ch_i
                for cc in range(4):
                    pc = (cc & 1) * 2 + ch_i
                    nc.vector.tensor_copy(
                        out=idx16b[cc * 32 : cc * 32 + 18, :].rearrange(
                            "r (l c) -> r l c", l=16),
                        in_=T2[cc * 32 : cc * 32 + 18, :].rearrange(
                            "r (c l) -> r c l", l=16).transpose([0, 2, 1]),
                    )
                    nc.sync.dma_start(
                        out=idx_dram[(cc // 2) * 18 : (cc // 2) * 18 + 18,
                                     pc * 1024 : pc * 1024 + 1024],
                        in_=idx16b[cc * 32 : cc * 32 + 18, :],
                    )
                # readback for the two pc ready after this pass
                for g in range(4):
                    for pcr in ([0, 2] if ch_i == 0 else [1, 3]):
                        srcv = bass.AP(
                            base_ap.tensor,
                            base_ap.offset + g * 9 * 4096 + pcr * 1024,
                            [[64, 16], [4096, 9], [1, 64]],
                        )
                        for rep in range(2):
                            p0 = g * 32 + rep * 16
                            nc.sync.dma_start(
                                out=idxt[p0 : p0 + 16,
                                         pcr * 576 : (pcr + 1) * 576].rearrange(
                                    "p (k c) -> p k c", k=9),
                                in_=srcv,
                            )

                # mask & corner weights
                nc.scalar.activation(out=OM[:], in_=OM[:], func=AF.Sigmoid)
                nc.vector.tensor_scalar(out=T0[:], in0=OY[:], scalar1=-1.0, scalar2=1.0,
                                        op0=ALU.mult, op1=ALU.add)
                nc.vector.tensor_scalar(out=T1[:], in0=OX[:], scalar1=-1.0, scalar2=1.0,
                                        op0=ALU.mult, op1=ALU.add)
                nc.vector.tensor_mul(T0[:], T0[:], OM[:])   # (1-fy)*m
                nc.vector.tensor_mul(OY[:], OY[:], OM[:])   # fy*m
                for qi, (ya, xa) in enumerate(((T0, T1), (T0, OX), (OY, T1), (OY, OX))):
                    dst_t = T2 if qi % 2 == 0 else T3
                    nc.vector.tensor_mul(dst_t[:], ya[:], xa[:])
                    for cc in range(4):
                        nc.scalar.activation(
                            out=wq[32 * qi : 32 * qi + 18,
                                   cc * 2048 + ch_i * 1024 :
                                   cc * 2048 + ch_i * 1024 + 1024],
                            in_=dst_t[cc * 32 : cc * 32 + 18, :], func=AF.Copy)

        nc.sync.dma_start(out=sel[:], in_=sel_ext[:])
        nc.sync.dma_start(out=wdup[:], in_=wdup_ext[:])
        nc.sync.dma_start(out=bdc_t[:], in_=bdc2_ext[:])

        # ======== phase 4+5: gathers (k-pairs) + modulate + matmul
        with (
            tc.tile_pool(name="g", bufs=2) as gpool,
            tc.tile_pool(name="h", bufs=3) as hpool,
            tc.tile_pool(name="o", bufs=1) as opool,
            tc.tile_pool(name="mp", bufs=4, space="PSUM") as mpsum,
            tc.tile_pool(name="op", bufs=1, space="PSUM") as opsum,
        ):
            _lib = nc.gpsimd.load_library(library_config.ap_gather)
            for _io in _iotas:
                tile.add_dep_helper(_lib.ins, _io.ins, reason="lib load after iotas")
            qtab_f32 = qtab[:].bitcast(DT.float32)
            outv = out_ext[:].rearrange("o h w -> o (h w)")

            KP = [(0, 1), (2, 3), (4, 5), (6, 7), (8,)]
            for pc in range(4):
                po0 = opsum.tile([128, 1024], DT.float32, tag="po0")
                po1 = opsum.tile([128, 1024], DT.float32, tag="po1")
                po = {0: po0, 1: po1}
                for kp in KP:
                    nk = len(kp)
                    g = gpool.tile([128, 2 * 1024 * 4], DT.float32, tag="g")
                    idx_sl = idxt[:, (pc * KF + kp[0]) * 64 :
                                  (pc * KF + kp[0] + nk) * 64]
                    _ga = nc.gpsimd.ap_gather(
                        g[:, 0 : nk * 1024 * 4], qtab_f32, idx_sl,
                        channels=128, num_elems=NBLK, d=4, num_idxs=nk * 1024,
                    )
                    tile.add_dep_helper(_ga.ins, _lib.ins, reason="gather after lib load")
                    gb = g[:].bitcast(DT.bfloat16)
                    for ki, k in enumerate(kp):
                        for sub in range(2):
                            col0 = pc * 1024 + sub * SUB
                            for qi in range(4):
                                mq = mpsum.tile([128, SUB], DT.float32, tag="mq")
                                for s2 in range(2):
                                    nc.tensor.matmul(
                                        out=mq[s2 * 64 : s2 * 64 + 64, :],
                                        lhsT=sel[:, (k * 4 + qi) * 64 :
                                                 (k * 4 + qi + 1) * 64],
                                        rhs=wq[:, s2 * 4096 + col0 :
                                               s2 * 4096 + col0 + SUB],
                                        start=True, stop=True,
                                    )
                                ht = hpool.tile([128, 2 * SUB], DT.bfloat16,
                                                tag=f"ht{qi}")
                                ht3 = ht[:].rearrange("p (w n) -> p w n", w=2)
                                goff = (ki * 1024 + sub * SUB) * 8
                                gq = bass.AP(
                                    gb.tensor, gb.offset + goff + qi,
                                    [gb.ap[0], [4, 2], [8, SUB]],
                                )
                                mq2 = bass.AP(
                                    mq[:].tensor, mq[:].offset,
                                    [mq[:].ap[0], [0, 2], [1, SUB]],
                                )
                                nc.vector.tensor_tensor(
                                    out=ht3, in0=mq2, in1=gq, op=ALU.mult)
                                for s2 in range(2):
                                    for w in range(2):
                                        nc.tensor.matmul(
                                            out=po[s2][:, sub * SUB : (sub + 1) * SUB],
                                            lhsT=wdup[s2 * 64 : s2 * 64 + 64,
                                                      (k * 2 + w) * 128 :
                                                      (k * 2 + w + 1) * 128],
                                            rhs=ht3[s2 * 64 : s2 * 64 + 64, w, :],
                                            start=(k == 0 and qi == 0 and w == 0),
                                            stop=(k == KF - 1 and qi == 3 and w == 1),
                                        )
                for s2 in range(2):
                    ot = opool.tile([128, 1024], DT.float32, tag="ot")
                    nc.vector.tensor_scalar(
                        out=ot[:], in0=po[s2][:], scalar1=bdc_t[:, 0:1],
                        scalar2=None, op0=ALU.add,
                    )
                    for h in range(2):
                        nc.sync.dma_start(
                            out=outv[:, h * NPIX + s2 * 4096 + pc * 1024 :
                                     h * NPIX + s2 * 4096 + (pc + 1) * 1024],
                            in_=ot[h * 64 : h * 64 + 64, :],
                        )


def _build_nc():
    _install_compat()
    nc = bass.Bass()
    x_ext = nc.declare_dram_parameter("x", [C, H, W], DT.float32, isOutput=False)
    lom_ext = nc.declare_dram_parameter("lom_h", [128, KF * 54], DT.bfloat16, isOutput=False)
    bom2_ext = nc.declare_dram_parameter("bom_h", [54, 1], DT.float32, isOutput=False)
    sel_ext = nc.declare_dram_parameter("sel_h", [128, KF * 4 * 64], DT.bfloat16, isOutput=False)
    wdup_ext = nc.declare_dram_parameter("wdup_h", [128, KF * 2 * 128], DT.bfloat16, isOutput=False)
    bdc2_ext = nc.declare_dram_parameter("bdc_h", [128, 1], DT.float32, isOutput=False)
    out_ext = nc.declare_dram_parameter("out", [O, H, W], DT.float32, isOutput=True)
    with tile.TileContext(nc) as tc:
        _emit(nc, tc, x_ext, out_ext, lom_ext, bom2_ext, sel_ext, wdup_ext, bdc2_ext)
    lower_extended_insts(nc)
    return nc


_NC_CACHE = None


def kernel(**inputs):
    global _NC_CACHE
    x = np.ascontiguousarray(inputs["x"], dtype=np.float32)
    w_om = np.ascontiguousarray(inputs["w_om"], dtype=np.float32)
    b_om = np.ascontiguousarray(inputs["b_om"], dtype=np.float32)
    w_dc = np.ascontiguousarray(inputs["w_dc"], dtype=np.float32)
    b_dc = np.ascontiguousarray(inputs["b_dc"], dtype=np.float32)

    if _NC_CACHE is None:
        _NC_CACHE = _build_nc()
    nc = _NC_CACHE

    in_maps = make_in_maps(x, w_om, b_om, w_dc, b_dc)
    res = run_bass_kernel_spmd(nc, in_maps, core_ids=list(range(NCORES)))
    return np.stack(
        [np.asarray(res.results[i]["out"]) for i in range(NCORES)]
    ).astype(np.float32)


def make_in_maps(x, w_om, b_om, w_dc, b_dc):
    import ml_dtypes

    bf16 = ml_dtypes.bfloat16
    # block-diagonal conv lhsT: [p=h*64+c, dd*54 + h*27 + role*9 + k]
    lom = np.zeros((128, KF, 54), np.float32)
    wom4 = w_om.reshape(3 * KF, C, KF)          # [o, c, dd]
    for h in range(2):
        for role in range(3):
            if role < 2:
                wsl = wom4[role:18:2]           # [k, c, dd]
            else:
                wsl = wom4[18:27]
            lom[h * 64 : h * 64 + 64, :, h * 27 + role * 9 : h * 27 + role * 9 + 9] = (
                wsl.transpose(1, 2, 0)          # [c, dd, k]
            )
    lom_h = np.ascontiguousarray(lom.reshape(128, KF * 54)).astype(bf16)
    bom2 = np.zeros((54, 1), np.float32)
    for h in range(2):
        bom2[h * 27 + 0 : h * 27 + 9, 0] = b_om[0:18:2]
        bom2[h * 27 + 9 : h * 27 + 18, 0] = b_om[1:18:2]
        bom2[h * 27 + 18 : h * 27 + 27, 0] = b_om[18:27]
    # selector: sel[r, (k*4+q)*64 + m] = 1 iff r == q*32 + (m//32)*9 + k
    r = np.arange(128)[:, None]
    cg = np.arange(KF * 4 * 64)[None, :]
    kq = cg // 64
    kcol = kq // 4
    qcol = kq % 4
    m = cg % 64
    hcol = m // 32
    sel_h = ((r == qcol * 32 + hcol * 9 + kcol).astype(np.float32)).astype(bf16)
    # wdup[(hk, c32), (k*2+w)*128 + hk*64 + o] = 2 * w_dc[o, w*32+c, k]
    wdc9 = w_dc.reshape(O, C, KF)
    wdup_np = np.zeros((64, KF, 2, 128), np.float32)
    for hk in range(2):
        for w in range(2):
            wdup_np[hk * 32 : hk * 32 + 32, :, w, hk * 64 : hk * 64 + 64] = (
                wdc9[:, w * 32 : w * 32 + 32, :].transpose(1, 2, 0)  # [c32, k, o]
            )
    wdup64 = 2.0 * wdup_np.reshape(64, KF * 2 * 128)
    wdup_h = np.ascontiguousarray(
        np.concatenate([wdup64, wdup64], axis=0)).astype(bf16)
    bdc_h = np.concatenate([b_dc, b_dc]).reshape(128, 1).astype(np.float32)

    shared = {
        "lom_h": lom_h, "bom_h": bom2, "sel_h": sel_h,
        "wdup_h": wdup_h, "bdc_h": bdc_h,
    }
    return [{"x": x[i], **shared} for i in range(NCORES)]


# revision 19
# speedup vs baseline: 1.2359x; 1.2359x over previous
"""Trainium2 Bass kernel for nn_AdaFeatBlock (modulated deformable-conv block).

Sharding: data-parallel over batch — 8 samples -> 8 NeuronCores, all weights
replicated; each core computes its sample end-to-end, host stacks outputs.

Per-core pipeline (one sample, x [64,128,128]):
  1. x -> bf16 padded layout x_sb: partition h*64+c; free = 76 rows
     (half-rows -6..69) x 138 cols (-4..133), zero borders.
  2. offset/mask 3x3 conv = 9 shifted matmuls, block-diagonal [128, 54]
     lhsT, PSUM-accumulated -> om [54, 8192] (per half: off_y k0..8 |
     off_x k0..8 | mask k0..8).
  3. Quad tables for ap_gather, CHANNEL-PAIRED (d=4): partition group
     g = st*2+h (st = row-half of the image half, h = image half) holds
     channel-pairs (c, c+32) of its stream; element = 16B = 2x2 pixel
     quad for c and c+32. Window per stream: 40 rows -> NBLK =
     4 classes x 20 x 68 = 5440 (fits ap_gather's 2^15-word limit).
     Because each 16-partition Q7 core uses its own index list, one
     gather SLOT serves FOUR samples (A-h0, A-h1, B-h0, B-h1):
     36864 slots instead of 73728. ap_gather is per-slot bound
     (~29ns d=2 vs ~31ns d=4 measured), so this halves the gather.
  4. Coordinate math in one pass on [128, 2048] tiles, partition
     P = 32*cc + h*9 + k; floor via the 2^23 trick; idx written in
     wrapped order, bounced via DRAM into per-group idxt streams;
     corner weights (x mask) -> wq [4q x 18(h,k) rows, 8192].
  5. Gathers as k-pairs per 1024-px chunk (20 calls); per (k, q, sub):
     2 selector matmuls broadcast wq rows -> mq [128, 512] PSUM; one
     DVE mult -> ht2 [128, 2(lo/hi), 512] bf16; deformable conv =
     K=64 lo/hi matmuls with 4-way block-diagonal channel weights,
     PSUM-accumulated over (9k x 4q x 2w) -> + b_dc -> out.

Bottleneck: ap_gather ~31ns/slot x 36864 ~ 1.14 ms. (SWDGE dma_gather
InstDMAGatherAnt crashes this firmware — mlp/attnmlp library loads fine
but the gather kills the exec unit; native indirect_dma_start works but
is 56ns/descriptor Q7-bound. Both measured on HW.)
"""

import numpy as np

import concourse.bass as bass
import concourse.tile as tile
from concourse import mybir
from concourse.bass_utils import run_bass_kernel_spmd
from concourse import library_config
from concourse.library_overlay import lower_extended_insts
from concourse.vector_clock import ScopedClock

AF = mybir.ActivationFunctionType
ALU = mybir.AluOpType
DT = mybir.dt

B, C, H, W = 8, 64, 128, 128
O = 64
K = 3
KF = 9
NCORES = 8
HALF = H // 2
NPIX = H * W // 2              # 8192 pixels per half
ROWS_ST = 76                   # stored rows per half (-6..69)
PITCH = 138                    # stored cols (-4..133)
RY = 20                        # y-block starts per parity per stream window
RX = 68                        # x-block starts per parity
NCLS = RY * RX                 # 1360
NBLK = 4 * NCLS                # 5440
NSLOT = 4 * KF * 1024          # 36864 slots (each = 4 samples)
SUB = 512


def _install_compat():
    """This walrus build accepts at most ONE sync-wait per instruction."""
    if getattr(tile.TileContext, "_adafeat_patched", False):
        return
    _orig_lower = tile.TileContext._lower_ordered_insts

    def _split_waits(nc, ordered):
        for insts in ordered.values():
            new_insts = []
            for inst in insts:
                si = inst.sync_info
                if si is not None and si.on_wait and len(si.on_wait) > 1:
                    waits = list(si.on_wait)
                    for w in waits[:-1]:
                        nop = mybir.InstNoOp(name=f"I-{nc.next_id()}", ins=[], outs=[])
                        nop.engine = inst.engine
                        nop.sync_info = mybir.SyncInfo(on_wait=[w], on_update=[])
                        new_insts.append(nop)
                    inst.sync_info = mybir.SyncInfo(
                        on_wait=[waits[-1]], on_update=list(si.on_update)
                    )
                new_insts.append(inst)
            insts[:] = new_insts

    def _lower_split(self, ordered):
        _split_waits(self.nc, ordered)
        return _orig_lower(self, ordered)

    def _drain_split(self, tick_clock, wait_clock):
        carrier = self.nc.sync.nop(nofuse=True)
        wait_clock.add_sem_waits(
            carrier.ins, ScopedClock({None: tick_clock.global_clock})
        )
        si = carrier.ins.sync_info
        if si is not None and si.on_wait and len(si.on_wait) > 1:
            waits = list(si.on_wait)
            carrier.ins.sync_info = mybir.SyncInfo(
                on_wait=waits[:1], on_update=list(si.on_update)
            )
            for w in waits[1:]:
                extra = self.nc.sync.nop(nofuse=True)
                extra.ins.sync_info = mybir.SyncInfo(on_wait=[w], on_update=[])
        self.nc.sync.drain()
        self.nc.all_engine_barrier()
        popped = self.nc._tile_sem_poison_stack.pop()
        assert popped is self._sem_poison
        self.nc.clear_and_free_semaphores(list(self.sems.allocated().values()))
        self.nc.all_engine_barrier()

    tile.TileContext._lower_ordered_insts = _lower_split
    tile.TileContext._drain_and_barrier = _drain_split
    tile.TileContext._adafeat_patched = True


def _emit(nc, tc, x_ext, out_ext, lom_ext, bom2_ext, sel_ext, wdup_ext, bdc2_ext):
    _iotas = []

    with tc.tile_pool(name="persist", bufs=1) as persist:
        wq = persist.tile([128, NPIX], DT.bfloat16)
        idxt = persist.tile([128, NSLOT // 16], DT.int16)
        wdup = persist.tile([128, KF * 2 * 128], DT.bfloat16)
        sel = persist.tile([128, KF * 4 * 64], DT.bfloat16)
        bdc_t = persist.tile([128, 1], DT.float32)
        # channel-paired quad tables: [128, NBLK, 4 f32] = [.., 8 bf16]
        qtab = persist.tile([128, NBLK * 8], DT.bfloat16)
        omp_cm = tc.tile_pool(name="omp", bufs=1)
        omp = omp_cm.__enter__()
        om = omp.tile([96, NPIX], DT.bfloat16)

        xpool = tc.tile_pool(name="xp", bufs=1)
        xp = xpool.__enter__()
        x_sb = xp.tile([128, ROWS_ST * PITCH], DT.bfloat16)

        x3 = lambda: x_sb[:].rearrange("p (r c) -> p r c", c=PITCH)

        # ======== phase 1: load x f32 via HWDGE, convert to bf16 on DVE
        nc.gpsimd.memset(x_sb[:], 0.0)
        nc.gpsimd.memset(wq[:], 0.0)
        xv = x_ext[:]
        with tc.tile_pool(name="xs", bufs=1) as xsp:
            xstage = xsp.tile([128, HALF * W], DT.float32)
            xs3 = xstage[:].rearrange("p (r c) -> p r c", c=W)
            for h in range(2):
                nc.sync.dma_start(
                    out=xstage[h * 64 : (h + 1) * 64, :],
                    in_=xv[:, h * HALF : (h + 1) * HALF, :].rearrange(
                        "c r w -> c (r w)"),
                )
            for h in range(2):
                r0 = max(0, h * HALF - 6)
                r1 = min(H - 1, h * HALF + 69)
                rloc = r0 - (h * HALF - 6)
                for sh in range(2):
                    s0 = max(r0, sh * HALF)
                    s1 = min(r1, sh * HALF + HALF - 1)
                    if s0 > s1:
                        continue
                    dl = rloc + (s0 - r0)
                    nc.vector.tensor_copy(
                        out=x3()[h * 64 : h * 64 + 64,
                                 dl : dl + (s1 - s0 + 1), 4 : 4 + W],
                        in_=xs3[sh * 64 : sh * 64 + 64,
                                s0 - sh * HALF : s1 - sh * HALF + 1, :],
                    )

        # xq staging for the quad-table fills: channel-paired, per-group
        # row windows, f32 straight from DRAM (8 HWDGE DMAs, overlaps conv)
        xqpool_cm = tc.tile_pool(name="xq", bufs=1)
        xqp = xqpool_cm.__enter__()
        xqf = xqp.tile([128, 2 * 44 * PITCH], DT.float32)
        x4 = xqf[:].rearrange("p (w r c) -> p w r c", w=2, c=PITCH)
        nc.vector.memset(xqf[:], 0.0)
        for g in range(4):
            st, h = g // 2, g % 2
            r_lo = h * 64 + 32 * st - 4
            wr0 = max(0, -r_lo)
            wr1 = min(44, 128 - r_lo)
            for w in range(2):
                nc.sync.dma_start(
                    out=x4[g * 32 : (g + 1) * 32, w, wr0:wr1, 4 : 4 + W],
                    in_=xv[w * 32 : w * 32 + 32, r_lo + wr0 : r_lo + wr1, :],
                )

        # ======== phase 2: offset/mask conv -> om [96, NPIX] bf16,
        # rows role*32 + h*9 + k (32-aligned role blocks so phase 3 can
        # load slices with plain DVE copies)
        with (
            tc.tile_pool(name="convw", bufs=1) as convw,
            tc.tile_pool(name="convp", bufs=2, space="PSUM") as convp,
        ):
            lhsT_om = convw.tile([128, KF * 96], DT.bfloat16)
            nc.sync.dma_start(out=lhsT_om[:], in_=lom_ext[:])
            bom_t = convw.tile([96, 1], DT.float32)
            nc.sync.dma_start(out=bom_t[:], in_=bom2_ext[:])

            rows_per_sub = SUB // W  # 4
            for cb in range(NPIX // SUB):
                pt = convp.tile([96, SUB], DT.float32, tag="cpt")
                r0 = cb * rows_per_sub
                for i, (dy, dx) in enumerate(
                    (dy, dx) for dy in range(3) for dx in range(3)
                ):
                    rhs = x3()[:, 6 + r0 + dy - 1 : 6 + r0 + dy - 1 + rows_per_sub,
                               3 + dx : 3 + dx + W]
                    nc.tensor.matmul(
                        out=pt[:], lhsT=lhsT_om[:, i * 96 : (i + 1) * 96], rhs=rhs,
                        start=(i == 0), stop=(i == 8),
                    )
                nc.vector.tensor_scalar(
                    out=om[:, cb * SUB : (cb + 1) * SUB], in0=pt[:],
                    scalar1=bom_t[:, 0:1], scalar2=None, op0=ALU.add,
                )

        # ======== quad table build: 32 full-width strided copies from xqf
        # qtab[g*32+i, blk, w*4 + qy*2+qx] = xq[g*32+i, w, 2*by+a+qy,
        #                                       2*bx + b + qx]
        q8 = qtab[:].rearrange("p (blk q) -> p blk q", q=8)
        opi = 0
        for w in range(2):
            for a in range(2):
                for b in range(2):
                    blk0 = (a * 2 + b) * NCLS
                    for qy in range(2):
                        for qx in range(2):
                            src = x4[:, w,
                                     a + qy : a + qy + 2 * (RY - 1) + 1 : 2,
                                     b + qx : b + qx + 2 * (RX - 1) + 1 : 2]
                            dst3 = q8[:, blk0 : blk0 + NCLS,
                                      w * 4 + qy * 2 + qx :
                                      w * 4 + qy * 2 + qx + 1]
                            dst = bass.AP(
                                dst3.tensor, dst3.offset,
                                [dst3.ap[0], [RX * 8, RY], [8, RX]],
                            )
                            if opi % 2 == 0:
                                nc.scalar.activation(out=dst, in_=src,
                                                     func=AF.Copy)
                            else:
                                nc.vector.tensor_copy(out=dst, in_=src)
                            opi += 1

        xqpool_cm.__exit__(None, None, None)
        xpool.__exit__(None, None, None)

        # ======== phase 3: coordinate math, two column-half passes on
        # [128, 1024] tiles; partition P = 32*cc + h*9 + k; stream st = cc//2
        with tc.tile_pool(name="math", bufs=1) as mpool:
            idx16b = mpool.tile([128, 1024], DT.int16)
            OY = mpool.tile([128, 1024], DT.float32)
            OX = mpool.tile([128, 1024], DT.float32)
            OM = mpool.tile([128, 1024], DT.float32)
            IOTY = mpool.tile([128, 1024], DT.float32)
            IOTX = mpool.tile([128, 1024], DT.float32)
            T0 = mpool.tile([128, 1024], DT.float32)
            T1 = mpool.tile([128, 1024], DT.float32)
            T2 = mpool.tile([128, 1024], DT.float32)
            T3 = mpool.tile([128, 1024], DT.float32)
            T4 = mpool.tile([128, 1024], DT.float32)
            cst = mpool.tile([128, 8], DT.float32)

            pidx = mpool.tile([128, 4], DT.float32)
            _iotas.append(nc.gpsimd.iota(pidx[:, 0:1], pattern=[[0, 1]],
                           channel_multiplier=1,
                           allow_small_or_imprecise_dtypes=True))
            _iotas.append(nc.gpsimd.iota(IOTY[:], pattern=[[1, 8], [0, 128]],
                           channel_multiplier=0,
                           allow_small_or_imprecise_dtypes=True))
            _iotas.append(nc.gpsimd.iota(IOTX[:], pattern=[[0, 8], [1, 128]],
                           channel_multiplier=0,
                           allow_small_or_imprecise_dtypes=True))
            P128 = pidx[:, 0:1]
            hh, kk, kh3, km3, ccv, stv, cy, cx = (cst[:, i : i + 1] for i in range(8))
            t_a = pidx[:, 1:2]
            # cc = P // 32 (values 0..3)
            nc.vector.tensor_scalar(out=ccv, in0=P128, scalar1=31.5, scalar2=None, op0=ALU.is_gt)
            nc.vector.tensor_scalar(out=t_a, in0=P128, scalar1=63.5, scalar2=None, op0=ALU.is_gt)
            nc.vector.tensor_add(ccv, ccv, t_a)
            nc.vector.tensor_scalar(out=t_a, in0=P128, scalar1=95.5, scalar2=None, op0=ALU.is_gt)
            nc.vector.tensor_add(ccv, ccv, t_a)
            # st = cc // 2
            nc.vector.tensor_scalar(out=stv, in0=ccv, scalar1=1.5, scalar2=None, op0=ALU.is_gt)
            # hk = P - 32*cc ; h = hk > 8.5 ; k = hk - 9*h
            nc.vector.tensor_scalar(out=t_a, in0=ccv, scalar1=-32.0, scalar2=None, op0=ALU.mult)
            nc.vector.tensor_add(t_a, t_a, P128)
            nc.vector.tensor_scalar(out=hh, in0=t_a, scalar1=8.5, scalar2=None, op0=ALU.is_gt)
            nc.vector.tensor_scalar(out=kk, in0=hh, scalar1=-9.0, scalar2=None, op0=ALU.mult)
            nc.vector.tensor_add(kk, kk, t_a)
            nc.vector.tensor_scalar(out=kh3, in0=kk, scalar1=2.5, scalar2=None, op0=ALU.is_gt)
            nc.vector.tensor_scalar(out=t_a, in0=kk, scalar1=5.5, scalar2=None, op0=ALU.is_gt)
            nc.vector.tensor_add(kh3, kh3, t_a)
            nc.vector.tensor_scalar(out=km3, in0=kh3, scalar1=-3.0, scalar2=None, op0=ALU.mult)
            nc.vector.tensor_add(km3, km3, kk)
            # cy = 16*cc - 32*st + kh3 + 515   (sy = off_y + rowiota + cy)
            nc.vector.tensor_scalar(out=cy, in0=ccv, scalar1=16.0, scalar2=None, op0=ALU.mult)
            nc.vector.tensor_scalar(out=t_a, in0=stv, scalar1=-32.0, scalar2=None, op0=ALU.mult)
            nc.vector.tensor_add(cy, cy, t_a)
            nc.vector.tensor_add(cy, cy, kh3)
            nc.vector.tensor_scalar(out=cy, in0=cy, scalar1=515.0, scalar2=None, op0=ALU.add)
            # cx = km3 + 515
            nc.vector.tensor_scalar(out=cx, in0=km3, scalar1=515.0, scalar2=None, op0=ALU.add)

            idx_dram = nc.dram_tensor("idx_scratch", [36, 4096], DT.int16)
            base_ap = idx_dram[:]
            nc.vector.memset(OY[:], 0.0)
            nc.vector.memset(OX[:], 0.0)
            nc.vector.memset(OM[:], 0.0)
            for ch_i in range(2):
                # load offsets/mask rows (DVE: keeps GPSIMD free for gathers)
                for cc in range(4):
                    cs = slice(cc * 2048 + ch_i * 1024, cc * 2048 + ch_i * 1024 + 1024)
                    for role, dstt in ((0, OY), (1, OX), (2, OM)):
                        nc.vector.tensor_copy(
                            out=dstt[cc * 32 : cc * 32 + 18, :],
                            in_=om[role * 32 : role * 32 + 18, cs],
                        )

                # sy = OY + rowiota + cy (+8 in second half); floor; frac
                nc.vector.tensor_add(T0[:], OY[:], IOTY[:])
                nc.vector.tensor_scalar(out=T0[:], in0=T0[:], scalar1=cy,
                                        scalar2=float(ch_i * 8), op0=ALU.add, op1=ALU.add)
                nc.vector.tensor_scalar(out=T2[:], in0=T0[:], scalar1=8388608.0,
                                        scalar2=-8388608.0, op0=ALU.add, op1=ALU.add)
                nc.vector.tensor_tensor(out=OY[:], in0=T2[:], in1=T0[:], op=ALU.is_gt)
                nc.vector.tensor_sub(T2[:], T2[:], OY[:])
                nc.vector.tensor_sub(OY[:], T0[:], T2[:])     # fy
                nc.vector.tensor_copy(out=T0[:], in_=T2[:])   # y0s
                # sx
                nc.vector.tensor_add(T1[:], OX[:], IOTX[:])
                nc.vector.tensor_scalar(out=T1[:], in0=T1[:], scalar1=cx,
                                        scalar2=None, op0=ALU.add)
                nc.vector.tensor_scalar(out=T2[:], in0=T1[:], scalar1=8388608.0,
                                        scalar2=-8388608.0, op0=ALU.add, op1=ALU.add)
                nc.vector.tensor_tensor(out=OX[:], in0=T2[:], in1=T1[:], op=ALU.is_gt)
                nc.vector.tensor_sub(T2[:], T2[:], OX[:])
                nc.vector.tensor_sub(OX[:], T1[:], T2[:])     # fx
                nc.vector.tensor_copy(out=T1[:], in_=T2[:])   # x0s

                # y0l = clamp(y0s-512, 0, 39); by = floor(y0l/2); a = y0l-2by
                nc.vector.tensor_scalar(out=T0[:], in0=T0[:], scalar1=-512.0,
                                        scalar2=None, op0=ALU.add)
                nc.vector.tensor_scalar(out=T0[:], in0=T0[:], scalar1=0.0, scalar2=39.0,
                                        op0=ALU.max, op1=ALU.min)
                nc.vector.tensor_scalar_mul(out=T0[:], in0=T0[:], scalar1=0.5)
                nc.vector.tensor_scalar(out=T3[:], in0=T0[:], scalar1=8388608.0,
                                        scalar2=-8388608.0, op0=ALU.add, op1=ALU.add)
                nc.vector.tensor_tensor(out=T2[:], in0=T3[:], in1=T0[:], op=ALU.is_gt)
                nc.vector.tensor_sub(T3[:], T3[:], T2[:])     # by
                nc.vector.tensor_sub(T2[:], T0[:], T3[:])     # a/2
                nc.vector.tensor_copy(out=T0[:], in_=T3[:])   # by
                # x0l = clamp(x0s-512, 0, 135); bx = floor(x0l/2); b = x0l-2bx
                nc.vector.tensor_scalar(out=T1[:], in0=T1[:], scalar1=-512.0,
                                        scalar2=None, op0=ALU.add)
                nc.vector.tensor_scalar(out=T1[:], in0=T1[:], scalar1=0.0, scalar2=135.0,
                                        op0=ALU.max, op1=ALU.min)
                nc.vector.tensor_scalar_mul(out=T1[:], in0=T1[:], scalar1=0.5)
                nc.vector.tensor_scalar(out=T4[:], in0=T1[:], scalar1=8388608.0,
                                        scalar2=-8388608.0, op0=ALU.add, op1=ALU.add)
                nc.vector.tensor_tensor(out=T3[:], in0=T4[:], in1=T1[:], op=ALU.is_gt)
                nc.vector.tensor_sub(T4[:], T4[:], T3[:])     # bx
                nc.vector.tensor_sub(T3[:], T1[:], T4[:])     # b/2
                nc.vector.tensor_copy(out=T1[:], in_=T4[:])   # bx

                # idx = a*2720 + b*1360 + by*68 + bx
                nc.vector.tensor_scalar_mul(out=T2[:], in0=T2[:], scalar1=float(2 * 2 * NCLS))
                nc.vector.tensor_scalar_mul(out=T3[:], in0=T3[:], scalar1=float(2 * NCLS))
                nc.vector.tensor_add(T2[:], T2[:], T3[:])
                nc.vector.tensor_scalar_mul(out=T0[:], in0=T0[:], scalar1=float(RX))
                nc.vector.tensor_add(T2[:], T2[:], T0[:])
                nc.vector.tensor_add(T2[:], T2[:], T1[:])

                # wrapped idx -> DRAM bounce; pc = (cc&1)*2 + ch_i
                for cc in range(4):
                    pc = (cc & 1) * 2 + ch_i
                    nc.vector.tensor_copy(
                        out=idx16b[cc * 32 : cc * 32 + 18, :].rearrange(
                            "r (l c) -> r l c", l=16),
                        in_=T2[cc * 32 : cc * 32 + 18, :].rearrange(
                            "r (c l) -> r c l", l=16).transpose([0, 2, 1]),
                    )
                    nc.sync.dma_start(
                        out=idx_dram[(cc // 2) * 18 : (cc // 2) * 18 + 18,
                                     pc * 1024 : pc * 1024 + 1024],
                        in_=idx16b[cc * 32 : cc * 32 + 18, :],
                    )
                # readback for the two pc ready after this pass
                for g in range(4):
                    for pcr in ([0, 2] if ch_i == 0 else [1, 3]):
                        srcv = bass.AP(
                            base_ap.tensor,
                            base_ap.offset + g * 9 * 4096 + pcr * 1024,
                            [[64, 16], [4096, 9], [1, 64]],
                        )
                        for rep in range(2):
                            p0 = g * 32 + rep * 16
                            nc.sync.dma_start(
                                out=idxt[p0 : p0 + 16,
                                         pcr * 576 : (pcr + 1) * 576].rearrange(
                                    "p (k c) -> p k c", k=9),
                                in_=srcv,
                            )

                # mask & corner weights
                nc.scalar.activation(out=OM[:], in_=OM[:], func=AF.Sigmoid)
                nc.vector.tensor_scalar(out=T0[:], in0=OY[:], scalar1=-1.0, scalar2=1.0,
                                        op0=ALU.mult, op1=ALU.add)
                nc.vector.tensor_scalar(out=T1[:], in0=OX[:], scalar1=-1.0, scalar2=1.0,
                                        op0=ALU.mult, op1=ALU.add)
                nc.vector.tensor_mul(T0[:], T0[:], OM[:])   # (1-fy)*m
                nc.vector.tensor_mul(OY[:], OY[:], OM[:])   # fy*m
                for qi, (ya, xa) in enumerate(((T0, T1), (T0, OX), (OY, T1), (OY, OX))):
                    dst_t = T2 if qi % 2 == 0 else T3
                    nc.vector.tensor_mul(dst_t[:], ya[:], xa[:])
                    for cc in range(4):
                        nc.scalar.activation(
                            out=wq[32 * qi : 32 * qi + 18,
                                   cc * 2048 + ch_i * 1024 :
                                   cc * 2048 + ch_i * 1024 + 1024],
                            in_=dst_t[cc * 32 : cc * 32 + 18, :], func=AF.Copy)

        omp_cm.__exit__(None, None, None)

        nc.sync.dma_start(out=sel[:], in_=sel_ext[:])
        nc.sync.dma_start(out=wdup[:], in_=wdup_ext[:])
        nc.sync.dma_start(out=bdc_t[:], in_=bdc2_ext[:])

        # ======== phase 4+5: gathers (k-pairs) + modulate + matmul
        with (
            tc.tile_pool(name="g", bufs=2) as gpool,
            tc.tile_pool(name="h", bufs=3) as hpool,
            tc.tile_pool(name="o", bufs=1) as opool,
            tc.tile_pool(name="mp", bufs=4, space="PSUM") as mpsum,
            tc.tile_pool(name="op", bufs=1, space="PSUM") as opsum,
        ):
            _lib = nc.gpsimd.load_library(library_config.ap_gather)
            for _io in _iotas:
                tile.add_dep_helper(_lib.ins, _io.ins, reason="lib load after iotas")
            qtab_f32 = qtab[:].bitcast(DT.float32)
            outv = out_ext[:].rearrange("o h w -> o (h w)")

            KP = [(0, 1), (2, 3), (4, 5), (6, 7), (8,)]
            for pc in range(4):
                po0 = opsum.tile([128, 1024], DT.float32, tag="po0")
                po1 = opsum.tile([128, 1024], DT.float32, tag="po1")
                po = {0: po0, 1: po1}
                for kp in KP:
                    nk = len(kp)
                    g = gpool.tile([128, 2 * 1024 * 4], DT.float32, tag="g")
                    idx_sl = idxt[:, (pc * KF + kp[0]) * 64 :
                                  (pc * KF + kp[0] + nk) * 64]
                    _ga = nc.gpsimd.ap_gather(
                        g[:, 0 : nk * 1024 * 4], qtab_f32, idx_sl,
                        channels=128, num_elems=NBLK, d=4, num_idxs=nk * 1024,
                    )
                    tile.add_dep_helper(_ga.ins, _lib.ins, reason="gather after lib load")
                    gb = g[:].bitcast(DT.bfloat16)
                    for ki, k in enumerate(kp):
                        for sub in range(2):
                            col0 = pc * 1024 + sub * SUB
                            for qi in range(4):
                                mq = mpsum.tile([128, SUB], DT.float32, tag="mq")
                                for s2 in range(2):
                                    nc.tensor.matmul(
                                        out=mq[s2 * 64 : s2 * 64 + 64, :],
                                        lhsT=sel[:, (k * 4 + qi) * 64 :
                                                 (k * 4 + qi + 1) * 64],
                                        rhs=wq[:, s2 * 4096 + col0 :
                                               s2 * 4096 + col0 + SUB],
                                        start=True, stop=True,
                                    )
                                ht = hpool.tile([128, 2 * SUB], DT.bfloat16,
                                                tag=f"ht{qi}")
                                ht3 = ht[:].rearrange("p (w n) -> p w n", w=2)
                                goff = (ki * 1024 + sub * SUB) * 8
                                gq = bass.AP(
                                    gb.tensor, gb.offset + goff + qi,
                                    [gb.ap[0], [4, 2], [8, SUB]],
                                )
                                mq2 = bass.AP(
                                    mq[:].tensor, mq[:].offset,
                                    [mq[:].ap[0], [0, 2], [1, SUB]],
                                )
                                nc.vector.tensor_tensor(
                                    out=ht3, in0=mq2, in1=gq, op=ALU.mult)
                                for s2 in range(2):
                                    for w in range(2):
                                        nc.tensor.matmul(
                                            out=po[s2][:, sub * SUB : (sub + 1) * SUB],
                                            lhsT=wdup[s2 * 64 : s2 * 64 + 64,
                                                      (k * 2 + w) * 128 :
                                                      (k * 2 + w + 1) * 128],
                                            rhs=ht3[s2 * 64 : s2 * 64 + 64, w, :],
                                            start=(k == 0 and qi == 0 and w == 0),
                                            stop=(k == KF - 1 and qi == 3 and w == 1),
                                        )
                for s2 in range(2):
                    ot = opool.tile([128, 1024], DT.float32, tag="ot")
                    nc.vector.tensor_scalar(
                        out=ot[:], in0=po[s2][:], scalar1=bdc_t[:, 0:1],
                        scalar2=None, op0=ALU.add,
                    )
                    for h in range(2):
                        nc.sync.dma_start(
                            out=outv[:, h * NPIX + s2 * 4096 + pc * 1024 :
                                     h * NPIX + s2 * 4096 + (pc + 1) * 1024],
                            in_=ot[h * 64 : h * 64 + 64, :],
                        )


def _build_nc():
    _install_compat()
    nc = bass.Bass()
    x_ext = nc.declare_dram_parameter("x", [C, H, W], DT.float32, isOutput=False)
    lom_ext = nc.declare_dram_parameter("lom_h", [128, KF * 96], DT.bfloat16, isOutput=False)
    bom2_ext = nc.declare_dram_parameter("bom_h", [96, 1], DT.float32, isOutput=False)
    sel_ext = nc.declare_dram_parameter("sel_h", [128, KF * 4 * 64], DT.bfloat16, isOutput=False)
    wdup_ext = nc.declare_dram_parameter("wdup_h", [128, KF * 2 * 128], DT.bfloat16, isOutput=False)
    bdc2_ext = nc.declare_dram_parameter("bdc_h", [128, 1], DT.float32, isOutput=False)
    out_ext = nc.declare_dram_parameter("out", [O, H, W], DT.float32, isOutput=True)
    with tile.TileContext(nc) as tc:
        _emit(nc, tc, x_ext, out_ext, lom_ext, bom2_ext, sel_ext, wdup_ext, bdc2_ext)
    lower_extended_insts(nc)
    return nc


_NC_CACHE = None


def kernel(**inputs):
    global _NC_CACHE
    x = np.ascontiguousarray(inputs["x"], dtype=np.float32)
    w_om = np.ascontiguousarray(inputs["w_om"], dtype=np.float32)
    b_om = np.ascontiguousarray(inputs["b_om"], dtype=np.float32)
    w_dc = np.ascontiguousarray(inputs["w_dc"], dtype=np.float32)
    b_dc = np.ascontiguousarray(inputs["b_dc"], dtype=np.float32)

    if _NC_CACHE is None:
        _NC_CACHE = _build_nc()
    nc = _NC_CACHE

    in_maps = make_in_maps(x, w_om, b_om, w_dc, b_dc)
    res = run_bass_kernel_spmd(nc, in_maps, core_ids=list(range(NCORES)))
    return np.stack(
        [np.asarray(res.results[i]["out"]) for i in range(NCORES)]
    ).astype(np.float32)


def make_in_maps(x, w_om, b_om, w_dc, b_dc):
    import ml_dtypes

    bf16 = ml_dtypes.bfloat16
    # block-diagonal conv lhsT: [p=h*64+c, dd*54 + h*27 + role*9 + k]
    lom = np.zeros((128, KF, 96), np.float32)
    wom4 = w_om.reshape(3 * KF, C, KF)          # [o, c, dd]
    for h in range(2):
        for role in range(3):
            if role < 2:
                wsl = wom4[role:18:2]           # [k, c, dd]
            else:
                wsl = wom4[18:27]
            r0 = role * 32 + h * 9
            lom[h * 64 : h * 64 + 64, :, r0 : r0 + 9] = (
                wsl.transpose(1, 2, 0)          # [c, dd, k]
            )
    lom_h = np.ascontiguousarray(lom.reshape(128, KF * 96)).astype(bf16)
    bom2 = np.zeros((96, 1), np.float32)
    for h in range(2):
        bom2[0 + h * 9 : 9 + h * 9, 0] = b_om[0:18:2]
        bom2[32 + h * 9 : 41 + h * 9, 0] = b_om[1:18:2]
        bom2[64 + h * 9 : 73 + h * 9, 0] = b_om[18:27]
    # selector: sel[r, (k*4+q)*64 + m] = 1 iff r == q*32 + (m//32)*9 + k
    r = np.arange(128)[:, None]
    cg = np.arange(KF * 4 * 64)[None, :]
    kq = cg // 64
    kcol = kq // 4
    qcol = kq % 4
    m = cg % 64
    hcol = m // 32
    sel_h = ((r == qcol * 32 + hcol * 9 + kcol).astype(np.float32)).astype(bf16)
    # wdup[(hk, c32), (k*2+w)*128 + hk*64 + o] = 2 * w_dc[o, w*32+c, k]
    wdc9 = w_dc.reshape(O, C, KF)
    wdup_np = np.zeros((64, KF, 2, 128), np.float32)
    for hk in range(2):
        for w in range(2):
            wdup_np[hk * 32 : hk * 32 + 32, :, w, hk * 64 : hk * 64 + 64] = (
                wdc9[:, w * 32 : w * 32 + 32, :].transpose(1, 2, 0)  # [c32, k, o]
            )
    wdup64 = 2.0 * wdup_np.reshape(64, KF * 2 * 128)
    wdup_h = np.ascontiguousarray(
        np.concatenate([wdup64, wdup64], axis=0)).astype(bf16)
    bdc_h = np.concatenate([b_dc, b_dc]).reshape(128, 1).astype(np.float32)

    shared = {
        "lom_h": lom_h, "bom_h": bom2, "sel_h": sel_h,
        "wdup_h": wdup_h, "bdc_h": bdc_h,
    }
    return [{"x": x[i], **shared} for i in range(NCORES)]
